# revision 7
# baseline (speedup 1.0000x reference)
"""Trainium2 Bass kernel for ChannelFeatures (channel-attention style module).

Computes, per batch element b:
    x_max[b] = max over (H,W) of features[b]          # (C,)
    x_avg[b] = mean over (H,W) of features[b]         # (C,)
    7 residual blocks (shared weights on both branches):
        x = prelu(W1[k] @ x + b1[k], a1[k]) + x
    scores[b] = sigmoid(x_max[b] + x_avg[b])          # (C,)
    out[b] = features[b] * scores[b]                  # broadcast over (H,W)

Sharding: pure data parallel over batch — 16 batch elements across 8 cores,
2 per core, weights replicated. No cross-core communication.

Device strategy per core (2 batch elements, each (65536, 64) fp32):
  The kernel is HBM-bound: 33.5 MB in + 33.5 MB out per core = 67 MB at
  ~358 GB/s => ~187 us floor. Everything is built so each HBM byte moves
  exactly once and the SDMA rings never starve:

  * Loads are SWDGE cast-DMAs (fp32 DRAM -> bf16 SBUF) into 32 independent
    0.5 MB cache tiles (16 MB) — the whole working set stays resident, no
    second read, and no load ever waits on a buffer.
  * Per-tile max: tensor_tensor max trees (bf16 2x DVE mode) instead of the
    1x-mode strided tensor_reduce; finished per batch by one more tree.
  * Per-tile sum: PE ones-matmul straight off the bf16 cache tiles,
    accumulated across the batch in PSUM (mean scale folded in at the end).
  * Scores broadcast to 128 partitions via a K=1 ones matmul on PE (no
    DRAM bounce on the critical path).
  * Pass 2: bf16 multiply (2x mode) into a small staging pool, then SWDGE
    cast-stores (bf16 SBUF -> fp32 DRAM). Output is bf16-rounded, which is
    ~0.2% relative — far inside the 2e-2 gate.
"""

import numpy as np
from contextlib import ExitStack, nullcontext

import concourse.bass as bass
import concourse.tile as tile
from concourse import masks, mybir
from concourse.bass_utils import run_bass_kernel_spmd

# Problem shapes (hardcoded per contract)
B, H, W, C = 16, 256, 256, 64
CONV_NUM = 7
NCORES = 8
BPC = B // NCORES          # batch elements per core
HW = H * W                 # 65536 spatial positions
P = 128                    # SBUF partitions
KF = 32                    # spatial rows per partition per tile
TILE_ROWS = P * KF         # 4096 spatial rows per tile
T = HW // TILE_ROWS        # 16 tiles per batch element
F32 = mybir.dt.float32
BF16 = mybir.dt.float16  # fp16: same 16-bit DVE/PE fast paths, 4x the mantissa

# test.py hooks: set PROFILE=True before calling kernel() to capture an NTFF
# trace; LAST_EXEC_NS then holds the max per-core HW execution time.
PROFILE = False
LAST_EXEC_NS = None
LAST_RESULTS = None


def _split_dma_waits(nc: bass.Bass) -> None:
    """The pinned walrus build rejects DMA instructions carrying more than one
    sync-wait ("Too many sync wait commands"). Tile's sem assignment is not
    transitively minimal, so slot-reuse instructions can get two waits
    (consumer release + WAW with the previous writer). Hoist all but the last
    wait onto wait-only EventSemaphore instructions on the same engine right
    before the instruction."""
    n = 0
    # num=200: outside every id Tile allocated (its end-of-kernel range-clear
    # covers the allocated block), so no collision with released Tile sems.
    dummy = nc.alloc_semaphore(name="wsplit_dummy", num=200)
    for fn in nc.m.functions:
        for blk in fn.blocks:
            new_insts = []
            for inst in blk.instructions:
                si = getattr(inst, "sync_info", None)
                if si is not None and len(si.on_wait) > 1:
                    for w in si.on_wait[:-1]:
                        ev = mybir.InstEventSemaphore(
                            name=f"WSPLIT-{n}", ins=[], outs=[]
                        )
                        n += 1
                        ev.engine = inst.engine
                        # Tick a dedicated dummy sem nobody waits on, so the
                        # simulator/race tooling (which require every
                        # instruction to carry an update) accept the carrier.
                        upd = mybir.SyncUpdate(
                            sync_type="semaphore",
                            id=dummy.num,
                            ant_name=dummy.name,
                            update_mode="sem-add-imm",
                            update_value=1,
                        )
                        ev.sync_info = mybir.SyncInfo(on_wait=[w], on_update=[upd])
                        new_insts.append(ev)
                    si.on_wait = [si.on_wait[-1]]
                new_insts.append(inst)
            blk.instructions = new_insts


def _build_nc() -> bass.Bass:
    nc = bass.Bass()
    feat = nc.declare_dram_parameter("features", [BPC, HW, C], F32, isOutput=False)
    wT = nc.declare_dram_parameter("wT", [C, CONV_NUM, C], F32, isOutput=False)
    bT = nc.declare_dram_parameter("bT", [C, CONV_NUM], F32, isOutput=False)
    aT = nc.declare_dram_parameter("aT", [C, CONV_NUM], F32, isOutput=False)
    out = nc.declare_dram_parameter("out", [BPC, HW, C], F32, isOutput=True)

    feat_t = feat[:].rearrange("b (t p k) c -> b t p k c", p=P, k=KF)
    out_t = out[:].rearrange("b (t p k) c -> b t p k c", p=P, k=KF)

    SEG = KF // 8            # 512-wide matmul segments per tile
    MAX = mybir.AluOpType.max

    with ExitStack() as ctx:
        tc = ctx.enter_context(tile.TileContext(nc))
        singles = ctx.enter_context(tc.tile_pool(name="singles", bufs=1))
        cache = ctx.enter_context(tc.tile_pool(name="cache", bufs=1))
        stgp = ctx.enter_context(tc.tile_pool(name="stgp", bufs=3))
        treep = ctx.enter_context(tc.tile_pool(name="treep", bufs=2))
        s2p = ctx.enter_context(tc.tile_pool(name="s2p", bufs=2))
        maxpp = ctx.enter_context(tc.tile_pool(name="maxpp", bufs=2))
        sctp = ctx.enter_context(tc.tile_pool(name="sctp", bufs=2))
        small = ctx.enter_context(tc.tile_pool(name="small", bufs=2))
        psum = ctx.enter_context(tc.tile_pool(name="psum", bufs=1, space="PSUM"))
        psum2 = ctx.enter_context(tc.tile_pool(name="psum2", bufs=2, space="PSUM"))

        # Constants (HWDGE loads; the SWDGE/POOL queue stays clear for tiles)
        w_sb = singles.tile([C, CONV_NUM, C], F32)   # [c_in, k, c_out]
        nc.sync.dma_start(out=w_sb[:], in_=wT[:])
        b_sb = singles.tile([C, CONV_NUM], F32)      # [c, k]
        nc.sync.dma_start(out=b_sb[:], in_=bT[:])
        a_sb = singles.tile([C, CONV_NUM], F32)      # [c, k] (a1[k] per row)
        nc.sync.dma_start(out=a_sb[:], in_=aT[:])
        ones_col = singles.tile([P, 1], BF16)
        nc.vector.memset(ones_col[:], 1.0)
        ones_row = singles.tile([1, P], F32)
        nc.vector.memset(ones_row[:], 1.0)
        one_hw = singles.tile([1, 1], F32)
        nc.vector.memset(one_hw[:], 1.0 / HW)
        identity = singles.tile([P, P], F32)

        # [channel, branch(0=max,1=avg), batch]
        xvec = singles.tile([C, 2, BPC], F32)

        for b in range(BPC):
            # ---- Pass 1(b): cast-load all tiles, reduce as they land ----
            cached = []
            maxp = maxpp.tile([P, T, 4, C], BF16, tag="maxp")
            psum_s = psum2.tile([1, 8 * C], F32, tag="psum_s")
            for t in range(T):
                tl = cache.tile([P, KF, C], BF16, tag=f"c{b}_{t}")
                nc.gpsimd.dma_start(out=tl[:], in_=feat_t[b, t])
                cached.append(tl)
                # Later batches' reduce work is pushed back in the scheduler's
                # model clock (tile_wait_until) so the previous batch's pass-2
                # mul/cast chain is ordered AHEAD of it on DVE/PE/ACT: the cost
                # model thinks loads are fast, and without this it front-loads
                # batch b+1's tree ops, head-of-line-blocking the multiplies
                # whose stores should overlap batch b+1's load stream.
                with tc.tile_wait_until(b) if b else nullcontext():
                    # per-tile max tree 32 -> 16 -> 8 -> 4 rows (2x TT mode)
                    tr = treep.tile([P, 16, C], BF16, tag="tree")
                    nc.vector.tensor_tensor(tr[:], tl[:, :16], tl[:, 16:], MAX)
                    nc.vector.tensor_tensor(tr[:, :8], tr[:, :8], tr[:, 8:], MAX)
                    nc.vector.tensor_tensor(maxp[:, t], tr[:, :4], tr[:, 4:8], MAX)
                    # sum: PE ones-matmul off the bf16 tile, PSUM-accumulated
                    # over the batch; (row, channel) mix folded at the end.
                    sv = tl[:].rearrange("p (s r) c -> p s (r c)", s=SEG)
                    for seg in range(SEG):
                        nc.tensor.matmul(
                            psum_s[:],
                            ones_col[:],
                            sv[:, seg],
                            start=(t == 0 and seg == 0),
                            stop=(t == T - 1 and seg == SEG - 1),
                        )
            if b == 0:
                # after the b0 load triggers are queued so it doesn't delay
                # them (make_identity runs on the gpsimd engine)
                masks.make_identity(nc, identity[:])

            # final max tree over (P, T*4, C), then cross-partition via PE
            # transpose + DVE reduce
            mview = maxp[:].rearrange("p t r c -> p (t r) c")
            s2t = s2p.tile([P, 32, C], BF16, tag="s2")
            nc.vector.tensor_tensor(s2t[:], mview[:, :32], mview[:, 32:], MAX)
            nc.vector.tensor_tensor(s2t[:, :16], s2t[:, :16], s2t[:, 16:], MAX)
            nc.vector.tensor_tensor(s2t[:, :8], s2t[:, :8], s2t[:, 8:16], MAX)
            nc.vector.tensor_tensor(s2t[:, :4], s2t[:, :4], s2t[:, 4:8], MAX)
            nc.vector.tensor_tensor(s2t[:, :2], s2t[:, :2], s2t[:, 2:4], MAX)
            maxr = small.tile([P, C], F32, tag="maxr")
            nc.vector.tensor_tensor(maxr[:], s2t[:, 0], s2t[:, 1], MAX)
            mt = psum.tile([C, P], F32, tag="mt")
            nc.tensor.transpose(mt[:], maxr[:], identity[:])
            nc.vector.reduce_max(
                out=xvec[:, 0, b : b + 1], in_=mt[:], axis=mybir.AxisListType.X
            )
            # fold (row, channel) mix: (1, C, 8) reduce -> (1, C)
            srow = small.tile([1, C], F32, tag="srow")
            nc.vector.reduce_sum(
                out=srow[:],
                in_=psum_s[:].rearrange("p (s c) -> p c s", c=C),
                axis=mybir.AxisListType.X,
            )
            # transpose row->column via K=1 matmul, folding the 1/HW scale
            av = psum.tile([C, 1], F32, tag="av")
            nc.tensor.matmul(av[:], srow[:], one_hw[:], start=True, stop=True)
            nc.vector.tensor_copy(xvec[:, 1, b : b + 1], av[:])

            # ---- Recurrence(b): 7 residual PReLU blocks on (C, 2) ----
            # Kept entirely on PE+DVE (bias add folded into the tensor_scalar
            # ops) to minimize cross-engine hops on this serial chain.
            xf = xvec[:, :, b]  # (C, 2): cols = (max, avg)
            for k in range(CONV_NUM):
                y = psum.tile([C, 2], F32, tag="y")
                nc.tensor.matmul(y[:], w_sb[:, k, :], xf, start=True, stop=True)
                pos = small.tile([C, 2], F32, tag="pos")
                nc.vector.tensor_scalar(
                    pos[:], y[:], b_sb[:, k : k + 1], 0.0,
                    mybir.AluOpType.add, mybir.AluOpType.max,
                )
                zmin = small.tile([C, 2], F32, tag="zmin")
                nc.vector.tensor_scalar(
                    zmin[:], y[:], b_sb[:, k : k + 1], 0.0,
                    mybir.AluOpType.add, mybir.AluOpType.min,
                )
                neg = small.tile([C, 2], F32, tag="neg")
                nc.vector.tensor_scalar_mul(neg[:], zmin[:], a_sb[:, k : k + 1])
                pn = small.tile([C, 2], F32, tag="pn")
                nc.vector.tensor_add(pn[:], pos[:], neg[:])
                xn = small.tile([C, 2], F32, tag="xn")
                nc.vector.tensor_add(xn[:], pn[:], xf)
                xf = xn[:]

            # scores(b) = sigmoid(x_max + x_avg): (C, 1)
            ssum = small.tile([C, 1], F32, tag="ssum")
            nc.vector.tensor_add(ssum[:], xf[:, 0:1], xf[:, 1:2])
            scores = small.tile([C, 1], F32, tag="scores")
            nc.scalar.activation(
                out=scores[:], in_=ssum[:], func=mybir.ActivationFunctionType.Sigmoid
            )
            # broadcast to all partitions on-chip: (C,1) -T-> (1,C), then a
            # K=1 ones matmul fans it out to (P, C); widen to a full bf16
            # (P, KF, C) tile so the pass-2 muls see dense step-1 operands.
            sc_t = psum.tile([1, C], F32, tag="sc_t")
            nc.tensor.transpose(sc_t[:], scores[:], identity[:C, :C])
            sc_sb = small.tile([1, C], F32, tag="sc_sb")
            nc.vector.tensor_copy(sc_sb[:], sc_t[:])
            bc_ps = psum.tile([P, C], F32, tag="bc")
            nc.tensor.matmul(bc_ps[:], ones_row[:], sc_sb[:], start=True, stop=True)
            bcb = small.tile([P, C], BF16, tag="bcb")
            nc.vector.tensor_copy(bcb[:], bc_ps[:])
            sct = sctp.tile([P, KF, C], BF16, tag="sct")
            nc.vector.tensor_copy(
                sct[:], bcb[:].unsqueeze(1).to_broadcast([P, KF, C])
            )

            # ---- Pass 2(b): 16-bit multiply in place, ACT widens to fp32
            # staging, stores ride the HWDGE (SP) ring so the read stream
            # (SWDGE ring) and write stream overlap at the SDMA engines.
            for t in range(T):
                nc.vector.tensor_mul(cached[t][:], cached[t][:], sct[:])
                stg = stgp.tile([P, KF, C], F32, tag="stg")
                nc.scalar.copy(out=stg[:], in_=cached[t][:])
                nc.sync.dma_start(out=out_t[b, t], in_=stg[:])

    _split_dma_waits(nc)
    return nc


def _prep_inputs(features, W1, b1, a1):
    feats = np.ascontiguousarray(features, dtype=np.float32).reshape(B, HW, C)
    # lhsT layout: wT[c_in, k, c_out] = W1[k, c_out, c_in]
    wT = np.ascontiguousarray(np.transpose(np.asarray(W1, np.float32), (2, 0, 1)))
    bT = np.ascontiguousarray(np.asarray(b1, np.float32).T)            # (C, 7)
    aT = np.ascontiguousarray(
        np.broadcast_to(np.asarray(a1, np.float32), (C, CONV_NUM))
    )
    return feats, wT, bT, aT


def kernel(features, W1, b1, a1):
    global LAST_EXEC_NS
    feats, wT, bT, aT = _prep_inputs(features, W1, b1, a1)
    nc = _build_nc()
    in_maps = [
        {
            "features": feats[i * BPC : (i + 1) * BPC],
            "wT": wT,
            "bT": bT,
            "aT": aT,
        }
        for i in range(NCORES)
    ]
    import os

    res = run_bass_kernel_spmd(
        nc,
        in_maps,
        list(range(NCORES)),
        trace=PROFILE,
        tmpdir=os.environ.get("BASS_TMPDIR"),
    )
    global LAST_RESULTS
    LAST_RESULTS = res
    LAST_EXEC_NS = res.exec_time_ns
    out = np.concatenate(
        [res.results[i]["out"].reshape(BPC, H, W, C) for i in range(NCORES)], axis=0
    )
    return out
